# revision 21
# baseline (speedup 1.0000x reference)
"""Causal self-attention with RoPE on 8 Trainium2 NeuronCores.

Sharding: tensor-parallel over heads (2 heads/core). Each core computes
QKV for its 2 heads over all tokens, runs attention, then multiplies its
128 rows of W_proj to produce a full-size PARTIAL output; the host sums
the 8 partials (no on-device collective at all).

Layout tricks:
  - x is pre-transposed and cast to bf16 on the host, so the QKV matmul
    contraction dim (D) is already on partitions — no on-device
    transposes of x.
  - V^T (the natural QKV matmul output) is moved to token-major layout
    with the DMA crossbar transpose (2-byte dtype).
  - Attention probs carry a 65th "ones" column through the AV matmul so
    softmax denominators come out of the PE for free.
  - bf16 for all big matmuls except QK^T/proj operands that stay
    fp32r-accuracy-critical... (QK inputs are bf16 rope outputs; scores
    accumulate in fp32 PSUM; projection runs in fp32r.)
"""

import math

import numpy as np
import ml_dtypes

import concourse.bass as bass
import concourse.mybir as mybir
import concourse.tile as tile
from concourse import bacc
from concourse.bass_utils import run_bass_kernel_spmd

# Problem shape (hardcoded per contest rules).
B, T, D = 2, 2048, 1024
H, DH = 16, 64
ROPE_BASE = 10000.0
N_CORES = 8
P = 128
N_STRIPS = T // 512                    # 4 strips of 512 tokens per batch
TOK = B * T                            # 4096 flat tokens
DC = D // P                            # 8 contraction chunks

FP32 = mybir.dt.float32
FP32R = mybir.dt.float32r
BF16 = mybir.dt.bfloat16
AF = mybir.ActivationFunctionType
ALU = mybir.AluOpType

SCALE = 1.0 / math.sqrt(DH)


def _emit_body(nc, tc, d, consts):
    """One full forward pass; emitted `reps` times for slope timing."""
    dram = consts["dram"]
    dram = consts["dram"]
    pm, pss, pyt, po = consts["pm"], consts["pss"], consts["pyt"], consts["po"]
    w_sb, wp_sb = consts["w_sb"], consts["wp_sb"]
    cos_sb, sin_sb = consts["cos_sb"], consts["sin_sb"]
    pmat_sb = consts["pmat_sb"]

    with (
        tc.tile_pool(name="xp", bufs=3) as xp,
        tc.tile_pool(name="qk", bufs=2) as qkp,
        tc.tile_pool(name="vp", bufs=2) as vp,
        tc.tile_pool(name="rp", bufs=2) as rp,
        tc.tile_pool(name="ptp", bufs=8) as ptp,
        tc.tile_pool(name="yp", bufs=2) as yp,
        tc.tile_pool(name="obp", bufs=2) as obp,
        tc.tile_pool(name="rbp", bufs=2) as rbp,
    ):
        # proj work queue: list of (y0, y1, va? ...) closure args pending
        pending_proj = []

        def emit_proj(ent):
            """Projection of one strip (delayed one strip for overlap)."""
            b, s, y0, y1 = ent
            for tt in range(4):
                t0g = b * T + s * 512 + tt * P      # global token row
                tsl = slice(s * 512 + tt * P, s * 512 + (tt + 1) * P)
                ob = obp.tile([P, D], BF16, tag="ob")
                for half in range(2):
                    osl = slice(half * 512, (half + 1) * 512)
                    pot = po.tile([P, 512], FP32, tag="po")
                    nc.tensor.matmul(pot[:], y0[:, tsl], wp_sb[0][:, osl],
                                     start=True, stop=False)
                    nc.tensor.matmul(pot[:], y1[:, tsl], wp_sb[1][:, osl],
                                     start=False, stop=True)
                    # PSUM -> SBUF bf16 convert, spread across engines
                    eng = (nc.vector, nc.gpsimd, nc.scalar, nc.gpsimd)[
                        (tt * 2 + half) % 4]
                    if eng is nc.scalar:
                        nc.scalar.activation(ob[:, osl], pot[:], AF.Copy)
                    else:
                        eng.tensor_copy(ob[:, osl], pot[:])
                nc.sync.dma_start(d["out"][t0g : t0g + P, :], ob[:])

        for b in range(B):
            qt = qkp.tile([P, T], BF16, tag="qt", name="qt")
            kt = qkp.tile([P, T], BF16, tag="kt", name="kt")
            # y^T for both heads, normalized; head h dims at partitions
            # 0-63 of y2n[h] (kept separate for lane alignment).
            y2u = yp.tile([P, T], FP32R, tag="y2u", name="y2u")
            y2t1 = yp.tile([64, T], FP32R, tag="y2t1", name="y2t1")
            va = vp.tile([P, T // P, 65], BF16, tag="va", name="va")
            vb = vp.tile([P, T // P, 65], BF16, tag="vb", name="vb")
            nc.gpsimd.memset(va[:, :, 64], 1.0)
            nc.gpsimd.memset(vb[:, :, 64], 1.0)

            for s in range(N_STRIPS):
                tok0 = b * T + s * 512
                # ---- load x^T strip (one DMA) ----
                xtb = xp.tile([P, DC, 512], BF16, tag="xtb", name="xtb")
                nc.sync.dma_start(
                    xtb[:],
                    d["xt"][:, tok0 : tok0 + 512].rearrange(
                        "(o p) t -> p o t", p=P),
                )

                # ---- QKV in two half-strips of 256 tokens ----
                for hs in range(2):
                    csl = slice(hs * 256, (hs + 1) * 256)   # cols in strip
                    sl = slice(s * 512 + hs * 256, s * 512 + hs * 256 + 256)
                    pmt = pm.tile([P, 3, 256], FP32, tag="pm")
                    for i in range(3):
                        for dc in range(DC):
                            nc.tensor.matmul(
                                pmt[:, i],
                                w_sb["qkv"[i]][:, dc],
                                xtb[:, dc, csl],
                                start=(dc == 0),
                                stop=(dc == DC - 1),
                            )
                    # rope on q and k: rotate_half is a constant signed
                    # permutation S; rot = S @ (q * |sin|-paired), done as a
                    # tiny PE matmul into the dead PSUM slot. All elementwise
                    # on DVE, q/k chains interleaved to hide latency.
                    a = [rp.tile([P, 256], FP32, tag=f"a{i}", name="a")
                         for i in range(2)]
                    c = [rp.tile([P, 256], BF16, tag=f"c{i}", name="c")
                         for i in range(2)]
                    for i in range(2):
                        nc.vector.tensor_tensor(
                            c[i][:], pmt[:, i], sin_sb[:, sl], ALU.mult)
                    for i in range(2):
                        nc.vector.tensor_tensor(
                            a[i][:], pmt[:, i], cos_sb[:, sl], ALU.mult)
                        nc.tensor.matmul(pmt[:, i], pmat_sb[:], c[i][:],
                                         start=True, stop=True)
                    for i, dst in ((0, qt), (1, kt)):
                        nc.vector.tensor_tensor(
                            dst[:, sl], a[i][:], pmt[:, i], ALU.add)
                    # V: convert V^T to bf16, crossbar-transpose to
                    # token-major, split per head.
                    vt = rp.tile([P, 256], BF16, tag="vt")
                    nc.gpsimd.tensor_copy(vt[:], pmt[:, 2])
                    vtile = vp.tile([P, 2, P], BF16, tag="vtile")
                    nc.sync.dma_start_transpose(vtile[:], vt[:])
                    kt0 = s * 4 + hs * 2
                    nc.gpsimd.tensor_copy(
                        va[:, kt0 : kt0 + 2, 0:64], vtile[:, :, 0:64])
                    nc.gpsimd.tensor_copy(
                        vb[:, kt0 : kt0 + 2, 0:64], vtile[:, :, 64:128])

                # ---- attention for q-strip s, both heads ----
                qsl = slice(s * 512, (s + 1) * 512)
                jmax = 4 * s + 3
                pyts = []
                pts = {}
                for h in range(2):
                    ph = 64 * h
                    # all QK + exp first (pt tiles buffered in SBUF)
                    for j in range(jmax + 1):
                        col0 = max(0, 128 * (j - 4 * s))
                        w = 512 - col0
                        pst = pss.tile([P, 512], FP32, tag="ps")
                        nc.tensor.matmul(
                            pst[:, 0:w],
                            kt[ph : ph + 64, j * P : (j + 1) * P],
                            qt[ph : ph + 64, s * 512 + col0 : (s + 1) * 512],
                            start=True,
                            stop=True,
                        )
                        pt = ptp.tile([P, 512], BF16, tag="pt")
                        nc.scalar.activation(
                            pt[:, 0:w], pst[:, 0:w], AF.Exp, scale=SCALE)
                        if j >= 4 * s:
                            # diagonal tile: keep where col - row >= 0
                            nc.gpsimd.affine_select(
                                out=pt[:, 0:P],
                                in_=pt[:, 0:P],
                                compare_op=ALU.is_ge,
                                fill=0.0,
                                base=0,
                                channel_multiplier=-1,
                                pattern=[[1, P]],
                            )
                        pts[(h, j)] = (pt, col0, w)
                    pyt_t = pyt.tile([65, 512], FP32, tag=f"pyt{h}", name="pyt_t")
                    pyts.append(pyt_t)
                for h in range(2):
                    v_h = va if h == 0 else vb
                    for j in range(jmax + 1):
                        pt, col0, w = pts[(h, j)]
                        nc.tensor.matmul(
                            pyts[h][:, col0:512],
                            v_h[:, j, :],
                            pt[:, 0:w],
                            start=(j == 0),
                            stop=(j == jmax),
                        )
                pts.clear()

                # ---- softmax denominators + normalize ----
                # reciprocal stays lane-aligned at partition 64; broadcast
                # down to partitions 0-63 via a DRAM round-trip.
                r2 = rbp.tile([65, 2, 512], FP32, tag="r2")
                for h in range(2):
                    nc.vector.reciprocal(r2[64:65, h], pyts[h][64:65, :])
                r_dram = dram.tile([2, 1, 512], FP32, tag="r_dram",
                                   name="r_dram")
                nc.sync.dma_start(r_dram[:, 0], r2[64, :, :])
                rbs = []
                for h in range(2):
                    rb = rbp.tile([64, 512], FP32, tag=f"rb{h}", name="rb")
                    nc.sync.dma_start(
                        rb[:], r_dram[h].to_broadcast((64, 512)))
                    rbs.append(rb)
                for h in range(2):
                    nc.vector.tensor_tensor(
                        y2n[h][:, qsl], pyts[h][0:64, :], rbs[h][:],
                        ALU.mult)

                # ---- delayed projection of previous strip ----
                pending_proj.append((b, s, y2n[0], y2n[1]))
                if len(pending_proj) > 1:
                    emit_proj(pending_proj.pop(0))

        while pending_proj:
            emit_proj(pending_proj.pop(0))


def _build_program(reps=1):
    nc = bacc.Bacc(None, target_bir_lowering=False, debug=False)

    d = {
        "xt": nc.dram_tensor("xt", [D, TOK], BF16, kind="ExternalInput"),
        "wq": nc.dram_tensor("wq", [D, P], BF16, kind="ExternalInput"),
        "wk": nc.dram_tensor("wk", [D, P], BF16, kind="ExternalInput"),
        "wv": nc.dram_tensor("wv", [D, P], BF16, kind="ExternalInput"),
        "wp": nc.dram_tensor("wp", [P, D], FP32, kind="ExternalInput"),
        "cos": nc.dram_tensor("cos", [P, T], FP32, kind="ExternalInput"),
        "sin": nc.dram_tensor("sin", [P, T], FP32, kind="ExternalInput"),
        "pmat": nc.dram_tensor("pmat", [P, P], BF16, kind="ExternalInput"),
        "out": nc.dram_tensor("out", [TOK, D], BF16, kind="ExternalOutput"),
    }

    with tile.TileContext(nc) as tc:
        with (
            tc.tile_pool(name="const", bufs=1) as cpool,
            tc.tile_pool(name="pm", bufs=1, space="PSUM") as pm,
            tc.tile_pool(name="pss", bufs=2, space="PSUM") as pss,
            tc.tile_pool(name="pyt", bufs=1, space="PSUM") as pyt,
            tc.tile_pool(name="po", bufs=2, space="PSUM") as po,
            tc.tile_pool(name="dram", bufs=1, space="DRAM") as dram,
        ):
            w_sb = {}
            for name in ("q", "k", "v"):
                w_sb[name] = cpool.tile(
                    [P, DC, P], BF16, tag=f"w{name}", name=f"w{name}")
                nc.scalar.dma_start(
                    w_sb[name][:],
                    d[f"w{name}"][:].rearrange("(o p) j -> p o j", p=P),
                )
            wp_sb = cpool.tile([P, D], FP32R, tag="wp", name="wp")
            nc.scalar.dma_start(wp_sb[:], d["wp"][:].bitcast(FP32R))
            cos_sb = cpool.tile([P, T], FP32)
            sin_sb = cpool.tile([P, T], FP32)
            nc.scalar.dma_start(cos_sb[:], d["cos"][:])
            nc.scalar.dma_start(sin_sb[:], d["sin"][:])
            pmat_sb = cpool.tile([P, P], BF16)
            nc.scalar.dma_start(pmat_sb[:], d["pmat"][:])

            consts = dict(
                dram=dram, pm=pm, pss=pss, pyt=pyt, po=po,
                w_sb=w_sb, wp_sb=wp_sb, cos_sb=cos_sb, sin_sb=sin_sb,
                pmat_sb=pmat_sb,
            )
            for _rep in range(reps):
                _emit_body(nc, tc, d, consts)

    nc.compile()
    return nc


_NC_CACHE = {}


def _get_program(reps=1):
    if reps not in _NC_CACHE:
        _NC_CACHE[reps] = _build_program(reps)
    return _NC_CACHE[reps]


def _host_tables():
    inv_freq = 1.0 / (ROPE_BASE ** (np.arange(0, DH, 2, dtype=np.float32) / DH))
    t = np.arange(T, dtype=np.float32)
    freqs = np.outer(t, inv_freq).astype(np.float32)  # (T, 32)
    cos_t = np.cos(freqs).T                           # (32, T)
    sin_t = np.sin(freqs).T
    cos = np.empty((P, T), np.float32)
    sin = np.empty((P, T), np.float32)
    for blk in range(4):
        # row j pairs with row pair(j); |sin| is the same for both, and the
        # rotate_half sign lives in the S permutation matrix instead.
        cos[blk * 32 : (blk + 1) * 32] = cos_t
        sin[blk * 32 : (blk + 1) * 32] = sin_t
    return cos, sin


def _host_pmat():
    """lhsT for the rotate_half matmul: out[j] = sum_p pmat[p, j] * c[p]."""
    pmat = np.zeros((P, P), np.float32)
    for j in range(P):
        jj = j % 64
        pair = j + 32 if jj < 32 else j - 32
        pmat[pair, j] = -1.0 if jj < 32 else 1.0
    return pmat.astype(ml_dtypes.bfloat16)


def make_in_maps(x, W_qkv, W_proj):
    x = np.asarray(x, np.float32).reshape(TOK, D)
    xt = np.ascontiguousarray(x.T).astype(ml_dtypes.bfloat16)
    W_qkv = np.asarray(W_qkv, np.float32)
    W_proj = np.asarray(W_proj, np.float32)
    cos, sin = _host_tables()
    pmat = _host_pmat()

    in_maps = []
    for c in range(N_CORES):
        j0 = c * P
        in_maps.append(
            {
                "xt": xt,
                "wq": np.ascontiguousarray(
                    W_qkv[:, j0 : j0 + P]).astype(ml_dtypes.bfloat16),
                "wk": np.ascontiguousarray(
                    W_qkv[:, D + j0 : D + j0 + P]).astype(ml_dtypes.bfloat16),
                "wv": np.ascontiguousarray(
                    W_qkv[:, 2 * D + j0 : 2 * D + j0 + P]).astype(
                        ml_dtypes.bfloat16),
                "wp": np.ascontiguousarray(W_proj[j0 : j0 + P, :]),
                "cos": cos,
                "sin": sin,
                "pmat": pmat,
            }
        )
    return in_maps


def kernel(x, W_qkv, W_proj):
    in_maps = make_in_maps(x, W_qkv, W_proj)
    nc = _get_program()
    res = run_bass_kernel_spmd(nc, in_maps, list(range(N_CORES)))
    return assemble([res.results[c]["out"] for c in range(N_CORES)])


def assemble(outs):
    acc = np.zeros((TOK, D), np.float32)
    for c in range(N_CORES):
        acc += np.asarray(outs[c]).astype(np.float32)
    return acc.reshape(B, T, D)


# revision 22
# speedup vs baseline: 2.8595x; 2.8595x over previous
"""Causal self-attention with RoPE on 8 Trainium2 NeuronCores.

Sharding: tensor-parallel over heads (2 heads/core). Each core computes
QKV for its 2 heads over all tokens, runs attention, then multiplies its
128 rows of W_proj to produce a full-size PARTIAL output; the host sums
the 8 partials (no on-device collective at all).

Layout tricks:
  - x is pre-transposed and cast to bf16 on the host, so the QKV matmul
    contraction dim (D) is already on partitions — no on-device
    transposes of x.
  - V^T (the natural QKV matmul output) is moved to token-major layout
    with the DMA crossbar transpose (2-byte dtype).
  - Attention probs carry a 65th "ones" column through the AV matmul so
    softmax denominators come out of the PE for free.
  - bf16 for all big matmuls except QK^T/proj operands that stay
    fp32r-accuracy-critical... (QK inputs are bf16 rope outputs; scores
    accumulate in fp32 PSUM; projection runs in fp32r.)
"""

import math

import numpy as np
import ml_dtypes

import concourse.bass as bass
import concourse.mybir as mybir
import concourse.tile as tile
from concourse import bacc
from concourse.bass_utils import run_bass_kernel_spmd

# Problem shape (hardcoded per contest rules).
B, T, D = 2, 2048, 1024
H, DH = 16, 64
ROPE_BASE = 10000.0
N_CORES = 8
P = 128
N_STRIPS = T // 512                    # 4 strips of 512 tokens per batch
TOK = B * T                            # 4096 flat tokens
DC = D // P                            # 8 contraction chunks

FP32 = mybir.dt.float32
FP32R = mybir.dt.float32r
BF16 = mybir.dt.bfloat16
AF = mybir.ActivationFunctionType
ALU = mybir.AluOpType

SCALE = 1.0 / math.sqrt(DH)


def _emit_body(nc, tc, d, consts):
    """One full forward pass; emitted `reps` times for slope timing."""
    dram = consts["dram"]
    dram = consts["dram"]
    pm, pss, pyt, po = consts["pm"], consts["pss"], consts["pyt"], consts["po"]
    w_sb, wp_sb = consts["w_sb"], consts["wp_sb"]
    cos_sb, sin_sb = consts["cos_sb"], consts["sin_sb"]
    pmat_sb = consts["pmat_sb"]

    with (
        tc.tile_pool(name="xp", bufs=2) as xp,
        tc.tile_pool(name="qk", bufs=2) as qkp,
        tc.tile_pool(name="vp", bufs=2) as vp,
        tc.tile_pool(name="rp", bufs=2) as rp,
        tc.tile_pool(name="ptp", bufs=8) as ptp,
        tc.tile_pool(name="yp", bufs=2) as yp,
        tc.tile_pool(name="obp", bufs=2) as obp,
        tc.tile_pool(name="rbp", bufs=2) as rbp,
    ):
        # proj work queue: list of (y0, y1, va? ...) closure args pending
        pending_proj = []

        def emit_proj(ent):
            """Projection of one strip (delayed one strip for overlap)."""
            b, s, y0, y1 = ent
            for tt in range(4):
                t0g = b * T + s * 512 + tt * P      # global token row
                tsl = slice(s * 512 + tt * P, s * 512 + (tt + 1) * P)
                ob = obp.tile([P, D], BF16, tag="ob")
                for half in range(2):
                    osl = slice(half * 512, (half + 1) * 512)
                    pot = po.tile([P, 512], FP32, tag="po")
                    nc.tensor.matmul(pot[:], y0[:, tsl], wp_sb[0][:, osl],
                                     start=True, stop=False)
                    nc.tensor.matmul(pot[:], y1[:, tsl], wp_sb[1][:, osl],
                                     start=False, stop=True)
                    # PSUM -> SBUF bf16 convert, spread across engines
                    eng = (nc.vector, nc.gpsimd, nc.scalar, nc.gpsimd)[
                        (tt * 2 + half) % 4]
                    if eng is nc.scalar:
                        nc.scalar.activation(ob[:, osl], pot[:], AF.Copy)
                    else:
                        eng.tensor_copy(ob[:, osl], pot[:])
                nc.sync.dma_start(d["out"][t0g : t0g + P, :], ob[:])

        for b in range(B):
            qt = qkp.tile([P, T], BF16, tag="qt", name="qt")
            kt = qkp.tile([P, T], BF16, tag="kt", name="kt")
            # y^T for both heads, normalized; head h dims at partitions
            # 0-63 of y2n[h] (kept separate for lane alignment).
            y2u = yp.tile([P, T], FP32R, tag="y2u", name="y2u")
            y2t1 = yp.tile([64, T], FP32R, tag="y2t1", name="y2t1")
            va = vp.tile([P, T // P, 65], BF16, tag="va", name="va")
            vb = vp.tile([P, T // P, 65], BF16, tag="vb", name="vb")
            nc.gpsimd.memset(va[:, :, 64], 1.0)
            nc.gpsimd.memset(vb[:, :, 64], 1.0)

            for s in range(N_STRIPS):
                tok0 = b * T + s * 512
                # ---- load x^T strip (one DMA) ----
                xtb = xp.tile([P, DC, 512], BF16, tag="xtb", name="xtb")
                nc.sync.dma_start(
                    xtb[:],
                    d["xt"][:, tok0 : tok0 + 512].rearrange(
                        "(o p) t -> p o t", p=P),
                )

                # ---- QKV in two half-strips of 256 tokens ----
                for hs in range(2):
                    csl = slice(hs * 256, (hs + 1) * 256)   # cols in strip
                    sl = slice(s * 512 + hs * 256, s * 512 + hs * 256 + 256)
                    pmt = pm.tile([P, 3, 256], FP32, tag="pm")
                    for i in range(3):
                        for dc in range(DC):
                            nc.tensor.matmul(
                                pmt[:, i],
                                w_sb["qkv"[i]][:, dc],
                                xtb[:, dc, csl],
                                start=(dc == 0),
                                stop=(dc == DC - 1),
                            )
                    # rope on q and k: rotate_half is a constant signed
                    # permutation S; rot = S @ (q * |sin|-paired), done as a
                    # tiny PE matmul into the dead PSUM slot. All elementwise
                    # on DVE, q/k chains interleaved to hide latency.
                    a = [rp.tile([P, 256], FP32, tag=f"a{i}", name="a")
                         for i in range(2)]
                    c = [rp.tile([P, 256], BF16, tag=f"c{i}", name="c")
                         for i in range(2)]
                    for i in range(2):
                        nc.vector.tensor_tensor(
                            c[i][:], pmt[:, i], sin_sb[:, sl], ALU.mult)
                    for i in range(2):
                        nc.vector.tensor_tensor(
                            a[i][:], pmt[:, i], cos_sb[:, sl], ALU.mult)
                        nc.tensor.matmul(pmt[:, i], pmat_sb[:], c[i][:],
                                         start=True, stop=True)
                    for i, dst in ((0, qt), (1, kt)):
                        nc.vector.tensor_tensor(
                            dst[:, sl], a[i][:], pmt[:, i], ALU.add)
                    # V: convert V^T to bf16, crossbar-transpose to
                    # token-major, split per head.
                    vt = rp.tile([P, 256], BF16, tag="vt")
                    nc.gpsimd.tensor_copy(vt[:], pmt[:, 2])
                    vtile = vp.tile([P, 2, P], BF16, tag="vtile")
                    nc.sync.dma_start_transpose(vtile[:], vt[:])
                    kt0 = s * 4 + hs * 2
                    nc.gpsimd.tensor_copy(
                        va[:, kt0 : kt0 + 2, 0:64], vtile[:, :, 0:64])
                    nc.gpsimd.tensor_copy(
                        vb[:, kt0 : kt0 + 2, 0:64], vtile[:, :, 64:128])

                # ---- attention for q-strip s, both heads ----
                qsl = slice(s * 512, (s + 1) * 512)
                jmax = 4 * s + 3
                pyts = []
                pts = {}
                for h in range(2):
                    ph = 64 * h
                    # all QK + exp first (pt tiles buffered in SBUF)
                    for j in range(jmax + 1):
                        col0 = max(0, 128 * (j - 4 * s))
                        w = 512 - col0
                        pst = pss.tile([P, 512], FP32, tag="ps")
                        nc.tensor.matmul(
                            pst[:, 0:w],
                            kt[ph : ph + 64, j * P : (j + 1) * P],
                            qt[ph : ph + 64, s * 512 + col0 : (s + 1) * 512],
                            start=True,
                            stop=True,
                        )
                        pt = ptp.tile([P, 512], BF16, tag="pt")
                        nc.scalar.activation(
                            pt[:, 0:w], pst[:, 0:w], AF.Exp, scale=SCALE)
                        if j >= 4 * s:
                            # diagonal tile: keep where col - row >= 0
                            nc.gpsimd.affine_select(
                                out=pt[:, 0:P],
                                in_=pt[:, 0:P],
                                compare_op=ALU.is_ge,
                                fill=0.0,
                                base=0,
                                channel_multiplier=-1,
                                pattern=[[1, P]],
                            )
                        pts[(h, j)] = (pt, col0, w)
                    pyt_t = pyt.tile([65, 512], FP32, tag=f"pyt{h}", name="pyt_t")
                    pyts.append(pyt_t)
                for h in range(2):
                    v_h = va if h == 0 else vb
                    for j in range(jmax + 1):
                        pt, col0, w = pts[(h, j)]
                        nc.tensor.matmul(
                            pyts[h][:, col0:512],
                            v_h[:, j, :],
                            pt[:, 0:w],
                            start=(j == 0),
                            stop=(j == jmax),
                        )
                pts.clear()

                # ---- softmax denominators + normalize ----
                # reciprocal stays lane-aligned at partition 64; broadcast
                # down to partitions 0-63 via a DRAM round-trip.
                r2 = rbp.tile([65, 2, 512], FP32, tag="r2")
                for h in range(2):
                    nc.vector.reciprocal(r2[64:65, h], pyts[h][64:65, :])
                r_dram = dram.tile([2, 1, 512], FP32, tag="r_dram",
                                   name="r_dram")
                nc.sync.dma_start(r_dram[:, 0], r2[64, :, :])
                rbs = []
                for h in range(2):
                    rb = rbp.tile([64, 512], FP32, tag=f"rb{h}", name="rb")
                    nc.sync.dma_start(
                        rb[:], r_dram[h].to_broadcast((64, 512)))
                    rbs.append(rb)
                for h in range(2):
                    nc.vector.tensor_tensor(
                        y2n[h][:, qsl], pyts[h][0:64, :], rbs[h][:],
                        ALU.mult)

                # ---- delayed projection of previous strip ----
                pending_proj.append((b, s, y2n[0], y2n[1]))
                if len(pending_proj) > 1:
                    emit_proj(pending_proj.pop(0))

        while pending_proj:
            emit_proj(pending_proj.pop(0))


def _build_program(reps=1):
    nc = bacc.Bacc(None, target_bir_lowering=False, debug=False)

    d = {
        "xt": nc.dram_tensor("xt", [D, TOK], BF16, kind="ExternalInput"),
        "wq": nc.dram_tensor("wq", [D, P], BF16, kind="ExternalInput"),
        "wk": nc.dram_tensor("wk", [D, P], BF16, kind="ExternalInput"),
        "wv": nc.dram_tensor("wv", [D, P], BF16, kind="ExternalInput"),
        "wp": nc.dram_tensor("wp", [P, D], FP32, kind="ExternalInput"),
        "cos": nc.dram_tensor("cos", [P, T], FP32, kind="ExternalInput"),
        "sin": nc.dram_tensor("sin", [P, T], FP32, kind="ExternalInput"),
        "pmat": nc.dram_tensor("pmat", [P, P], BF16, kind="ExternalInput"),
        "out": nc.dram_tensor("out", [TOK, D], BF16, kind="ExternalOutput"),
    }

    with tile.TileContext(nc) as tc:
        with (
            tc.tile_pool(name="const", bufs=1) as cpool,
            tc.tile_pool(name="pm", bufs=1, space="PSUM") as pm,
            tc.tile_pool(name="pss", bufs=2, space="PSUM") as pss,
            tc.tile_pool(name="pyt", bufs=1, space="PSUM") as pyt,
            tc.tile_pool(name="po", bufs=2, space="PSUM") as po,
            tc.tile_pool(name="dram", bufs=1, space="DRAM") as dram,
        ):
            w_sb = {}
            for name in ("q", "k", "v"):
                w_sb[name] = cpool.tile(
                    [P, DC, P], BF16, tag=f"w{name}", name=f"w{name}")
                nc.scalar.dma_start(
                    w_sb[name][:],
                    d[f"w{name}"][:].rearrange("(o p) j -> p o j", p=P),
                )
            wp_sb = cpool.tile([P, D], FP32R, tag="wp", name="wp")
            nc.scalar.dma_start(wp_sb[:], d["wp"][:].bitcast(FP32R))
            cos_sb = cpool.tile([P, T], FP32)
            sin_sb = cpool.tile([P, T], FP32)
            nc.scalar.dma_start(cos_sb[:], d["cos"][:])
            nc.scalar.dma_start(sin_sb[:], d["sin"][:])
            pmat_sb = cpool.tile([P, P], BF16)
            nc.scalar.dma_start(pmat_sb[:], d["pmat"][:])

            consts = dict(
                dram=dram, pm=pm, pss=pss, pyt=pyt, po=po,
                w_sb=w_sb, wp_sb=wp_sb, cos_sb=cos_sb, sin_sb=sin_sb,
                pmat_sb=pmat_sb,
            )
            for _rep in range(reps):
                _emit_body(nc, tc, d, consts)

    nc.compile()
    return nc


_NC_CACHE = {}


def _get_program(reps=1):
    if reps not in _NC_CACHE:
        _NC_CACHE[reps] = _build_program(reps)
    return _NC_CACHE[reps]


def _host_tables():
    inv_freq = 1.0 / (ROPE_BASE ** (np.arange(0, DH, 2, dtype=np.float32) / DH))
    t = np.arange(T, dtype=np.float32)
    freqs = np.outer(t, inv_freq).astype(np.float32)  # (T, 32)
    cos_t = np.cos(freqs).T                           # (32, T)
    sin_t = np.sin(freqs).T
    cos = np.empty((P, T), np.float32)
    sin = np.empty((P, T), np.float32)
    for blk in range(4):
        # row j pairs with row pair(j); |sin| is the same for both, and the
        # rotate_half sign lives in the S permutation matrix instead.
        cos[blk * 32 : (blk + 1) * 32] = cos_t
        sin[blk * 32 : (blk + 1) * 32] = sin_t
    return cos, sin


def _host_pmat():
    """lhsT for the rotate_half matmul: out[j] = sum_p pmat[p, j] * c[p]."""
    pmat = np.zeros((P, P), np.float32)
    for j in range(P):
        jj = j % 64
        pair = j + 32 if jj < 32 else j - 32
        pmat[pair, j] = -1.0 if jj < 32 else 1.0
    return pmat.astype(ml_dtypes.bfloat16)


def make_in_maps(x, W_qkv, W_proj):
    x = np.asarray(x, np.float32).reshape(TOK, D)
    xt = np.ascontiguousarray(x.T).astype(ml_dtypes.bfloat16)
    W_qkv = np.asarray(W_qkv, np.float32)
    W_proj = np.asarray(W_proj, np.float32)
    cos, sin = _host_tables()
    pmat = _host_pmat()

    in_maps = []
    for c in range(N_CORES):
        j0 = c * P
        in_maps.append(
            {
                "xt": xt,
                "wq": np.ascontiguousarray(
                    W_qkv[:, j0 : j0 + P]).astype(ml_dtypes.bfloat16),
                "wk": np.ascontiguousarray(
                    W_qkv[:, D + j0 : D + j0 + P]).astype(ml_dtypes.bfloat16),
                "wv": np.ascontiguousarray(
                    W_qkv[:, 2 * D + j0 : 2 * D + j0 + P]).astype(
                        ml_dtypes.bfloat16),
                "wp": np.ascontiguousarray(W_proj[j0 : j0 + P, :]),
                "cos": cos,
                "sin": sin,
                "pmat": pmat,
            }
        )
    return in_maps


def kernel(x, W_qkv, W_proj):
    in_maps = make_in_maps(x, W_qkv, W_proj)
    nc = _get_program()
    res = run_bass_kernel_spmd(nc, in_maps, list(range(N_CORES)))
    return assemble([res.results[c]["out"] for c in range(N_CORES)])


def assemble(outs):
    acc = np.zeros((TOK, D), np.float32)
    for c in range(N_CORES):
        acc += np.asarray(outs[c]).astype(np.float32)
    return acc.reshape(B, T, D)
